# revision 1
# baseline (speedup 1.0000x reference)
"""Bass/Trainium2 kernel for nn_BQAVariant (BQA: basis-weighted KV attention).

Reference computation (B=2, T=2048, D=768, H=12 q-heads, KH=4 KV basis
heads, HD=64):
  q = x@wq; k_basis = x@wk; v_basis = x@wv
  w = softmax(alpha); k/v = einsum('hj,btjd->bthd', w, {k,v}_basis)
  q,k = rmsnorm(rope(q,k)) * 1.2
  y = causal_sdpa(q, k, v, scale=HD**-0.5) @ wo

Sharding: 24 (batch, head) pairs over 8 cores -> core c handles batch c//4
and heads {3g, 3g+1, 3g+2} with g = c%4.  The alpha-softmax basis
combination is folded on the host into effective per-head wk/wv
(k_h = x @ sum_j w[h,j] wk[:, j*64:(j+1)*64]), so each core runs three
independent standard attention heads.  Each core emits its partial
c_proj output (transposed, [768, 2048]); the host sums the 4 partials
per batch.

Device kernel (per core), fp32 data / fp32 PSUM accumulation:
  A) QKV projections from pre-transposed xT [768, 2048] with a packed
     [768, 640] weight (q|k|v|pad so both matmul chunks have free dim
     >= 256); fused RoPE (3 DVE ops using host-prepared [cos|sin] and
     [-sin|cos] tables), RMSNorm (post-rope, x1.2 folded into the
     rsqrt), PE-transpose of Q,K into [64, T] per-head layout; V kept
     natural with a ones column appended (V') so the softmax denominator
     falls out of the PV matmul for free.
  B) Transposed flash attention per head: S^T[k,q] = K^T.T @ Q^T; since
     q,k are rms-normed, |s| <= 64*1.44/8 = 11.52, exp() cannot
     overflow, and the max-subtraction pass is skipped.  P^T =
     exp(S^T/8) (masked on diagonal k-tiles); psum_o[65, 512]
     accumulates V'^T @ P^T over k-tiles; row 64 is the softmax
     denominator.  Normalize via reciprocal + K=1 ones-matmul broadcast
     + one DVE multiply.
  C) partial^T = wo_c^T @ Y^T with the K=192 contraction split into 3
     per-head k-tiles of 64 (keeps every operand at partitions 0:64).
"""

import sys

sys.path.insert(0, "/opt/trn_rl_repo")

import numpy as np

import concourse.bass as bass
import concourse.tile as tile
from concourse import bacc, mybir
from concourse.bass import ts
from concourse.bass_utils import run_bass_kernel_spmd
from concourse.masks import make_identity

F32 = mybir.dt.float32
F32R = mybir.dt.float32r

B, T, D = 2, 2048, 768
H, KH, HD = 12, 4, 64
HPC = 3            # heads per core
NCORES = 8
EPS = 1e-6
QK = 1.2
NTT = T // 128     # 16 T-tiles
NKD = D // 128     # 6 contraction tiles for projections
QCH = 512          # q-chunk width in flash stage
NQC = T // QCH     # 4 q-chunks
WQKV = 640         # 192 q + 192 k + 192 v + 64 pad

# Matmul input dtype: float32r streams fp32 data through the PE's
# single-pass path when the moving free dim is >= 256 (4x fp32 rate).
MM_DT = F32R

# analysis knob: "all" | "a" (projections only) | "b" (attention+cproj only)
STAGES = "all"
# hardware-timing knob: repeat the whole body N times inside a For_i loop
# (difference timing: (wall[R] - wall[1]) / (R - 1) isolates kernel time
# from the ~100ms axon dispatch overhead)
REPEAT = 1


def build_nc():
    nc = bacc.Bacc(None, target_bir_lowering=False)

    xT = nc.declare_dram_parameter("xT", [D, T], MM_DT, isOutput=False)
    wqkv = nc.declare_dram_parameter("wqkv", [D, WQKV], MM_DT, isOutput=False)
    wo = nc.declare_dram_parameter("wo", [HPC * HD, D], MM_DT, isOutput=False)
    csn = nc.declare_dram_parameter("csn", [T, HD], F32, isOutput=False)
    scn = nc.declare_dram_parameter("scn", [T, HD], F32, isOutput=False)
    masks = nc.declare_dram_parameter("masks", [128, 128], F32, isOutput=False)
    outT = nc.declare_dram_parameter("outT", [D, T], F32, isOutput=True)

    with tile.TileContext(nc) as tc:
        with (
            tc.tile_pool(name="persist", bufs=1) as persist,
            tc.tile_pool(name="qkt", bufs=1) as qkt,
            tc.tile_pool(name="ropetmp", bufs=2) as ropetmp,
            tc.tile_pool(name="p_sb", bufs=3) as p_pool,
            tc.tile_pool(name="misc", bufs=2) as misc,
            tc.tile_pool(name="co_sb", bufs=2) as co_pool,
            # PSUM: 8 banks total, shared by both phases so they can overlap:
            # psqk/pp 1 + psv 1 + ptr 2 + ps_s 2 + po 2 = 8
            tc.tile_pool(name="proj_ps", bufs=1, space="PSUM") as proj_ps,
            tc.tile_pool(name="projv_ps", bufs=1, space="PSUM") as projv_ps,
            tc.tile_pool(name="tr_ps", bufs=1, space="PSUM") as tr_ps,
            tc.tile_pool(name="attn_ps", bufs=2, space="PSUM") as attn_ps,
            tc.tile_pool(name="o_ps", bufs=1, space="PSUM") as o_ps,
        ):
            # --- persistent SBUF tensors ---
            wqkv_sb = persist.tile([128, NKD, WQKV], MM_DT)
            wqkv_r = wqkv.rearrange("(k p) n -> p k n", p=128)
            for k in range(NKD):
                eng = nc.sync if k % 2 == 0 else nc.gpsimd
                eng.dma_start(out=wqkv_sb[:, k, :], in_=wqkv_r[:, k, :])

            wo_sb = [persist.tile([64, D], MM_DT, tag=f"wo{h}", name=f"wo_sb{h}")
                     for h in range(HPC)]
            for h in range(HPC):
                nc.sync.dma_start(out=wo_sb[h], in_=wo[h * 64:(h + 1) * 64, :])

            csn_sb = persist.tile([128, NTT, HD], F32)
            scn_sb = persist.tile([128, NTT, HD], F32)
            nc.sync.dma_start(out=csn_sb, in_=csn.rearrange("(i p) d -> p i d", p=128))
            nc.sync.dma_start(out=scn_sb, in_=scn.rearrange("(i p) d -> p i d", p=128))

            mask_sb = persist.tile([128, 128], F32)
            nc.sync.dma_start(out=mask_sb, in_=masks[:])

            ident = persist.tile([128, 128], F32)
            make_identity(nc, ident)

            ones_sb = persist.tile([128, 64], F32)
            nc.vector.memset(ones_sb, 1.0)
            ones_r = persist.tile([128, 64], MM_DT)
            nc.scalar.copy(ones_r, ones_sb)

            eps_sb = persist.tile([128, 1], F32)
            nc.vector.memset(eps_sb, EPS / (QK * QK))

            # V' with ones column: [128, i, h, 65].  memset can't write
            # f32r, so fill the ones columns with one strided ACT copy.
            vp_sb = persist.tile([128, NTT, HPC, 65], MM_DT)
            nc.scalar.copy(
                vp_sb[:, :, :, 64:65],
                ones_sb[:, 0:1].unsqueeze(1).broadcast_to([128, NTT, HPC, 1]))

            # Q^T / K^T / Y^T: packed [64, head, T], partitions 0:64
            qt_all = qkt.tile([64, HPC, T], MM_DT)
            kt_all = qkt.tile([64, HPC, T], MM_DT)
            yt_all = qkt.tile([64, HPC, T], MM_DT)

            xT_sb = persist.tile([128, NKD, T], MM_DT)
            xT_r = xT.rearrange("(k p) t -> p k t", p=128)
            for k in range(NKD):
                eng = nc.sync if k % 2 == 0 else nc.gpsimd
                eng.dma_start(out=xT_sb[:, k, :], in_=xT_r[:, k, :])

            def stage_a(i):
                isl = ts(i, 128)
                ps_qk = proj_ps.tile([128, 384], F32, tag="psqk", name="ps_qk")
                ps_v = projv_ps.tile([128, 256], F32, tag="psv", name="ps_v")
                for k in range(NKD):
                    lhsT = xT_sb[:, k, isl]
                    st = dict(start=(k == 0), stop=(k == NKD - 1))
                    nc.tensor.matmul(ps_qk, lhsT, wqkv_sb[:, k, 0:384], **st)
                    nc.tensor.matmul(ps_v, lhsT, wqkv_sb[:, k, 384:640], **st)

                # V -> V': one strided DVE copy into the 65-wide slots
                nc.vector.tensor_copy(
                    vp_sb[:, i, :, 0:64],
                    ps_v[:, 0:192].rearrange("p (h e) -> p h e", e=64))

                # rope for q and k; sumsq batched into one [128, 6] tile
                ssum = ropetmp.tile([128, 2, HPC], F32, tag="ssum", name="ssum")
                ropeds = []
                for which in range(2):
                    ps = ps_qk[:, which * 192:(which + 1) * 192]
                    hv = ps.rearrange("p (h two e) -> p two h e", two=2, e=32)
                    x1 = hv[:, 0:1, :, :].broadcast_to([128, 2, HPC, 32])
                    x2 = hv[:, 1:2, :, :].broadcast_to([128, 2, HPC, 32])
                    cs = csn_sb[:, i, :].rearrange("p (two e) -> p two e", two=2) \
                        .unsqueeze(2).broadcast_to([128, 2, HPC, 32])
                    sc = scn_sb[:, i, :].rearrange("p (two e) -> p two e", two=2) \
                        .unsqueeze(2).broadcast_to([128, 2, HPC, 32])
                    t1 = ropetmp.tile([128, 2, HPC, 32], F32,
                                      tag=f"t1{which}", name="t1")
                    t2 = ropetmp.tile([128, 2, HPC, 32], F32,
                                      tag=f"t2{which}", name="t2")
                    nc.vector.tensor_mul(t1, x1, cs)
                    nc.vector.tensor_mul(t2, x2, sc)
                    roped = ropetmp.tile([128, HPC, 2, 32], F32,
                                         tag=f"roped{which}", name="roped")
                    rview = roped.rearrange("p h two e -> p two h e")
                    nc.vector.tensor_add(rview, t1, t2)
                    rflat = roped.rearrange("p h two e -> p h (two e)")
                    sq = ropetmp.tile([128, HPC, HD], F32,
                                      tag=f"sq{which}", name="sq")
                    nc.vector.tensor_mul(sq, rflat, rflat)
                    nc.vector.reduce_sum(ssum[:, which, :], sq,
                                         axis=mybir.AxisListType.X)
                    ropeds.append(rflat)

                # one sqrt + one reciprocal for q and k together
                rstd = ropetmp.tile([128, 2, HPC], F32, tag="rstd", name="rstd")
                nc.scalar.activation(rstd, ssum,
                                     mybir.ActivationFunctionType.Sqrt,
                                     bias=eps_sb, scale=1.0 / (HD * QK * QK))
                nc.vector.reciprocal(rstd, rstd)

                for which, dst in enumerate((qt_all, kt_all)):
                    normed = ropetmp.tile([128, HPC, HD], F32,
                                          tag=f"normed{which}", name="normed")
                    nc.vector.tensor_mul(
                        normed, ropeds[which],
                        rstd[:, which, :].unsqueeze(2)
                            .broadcast_to([128, HPC, HD]))
                    # 3 PE transposes into one PSUM bank, one strided eviction
                    ptr = tr_ps.tile([64, HPC, 128], F32, tag="ptr", name="ptr")
                    for h in range(HPC):
                        nc.tensor.transpose(ptr[:, h, :], normed[:, h, :], ident)
                    nc.vector.tensor_copy(dst[:, :, isl], ptr)

            def attention(qc):
                qsl = ts(qc, QCH)
                njt = 4 * qc + 4  # causal: k-tiles overlapping this q-chunk
                for h in range(HPC):
                    po = o_ps.tile([65, QCH], F32, tag="po", name="po")
                    for j0 in range(0, njt, 2):
                        pair = [j for j in (j0, j0 + 1) if j < njt]
                        # ps_s spans 2 PSUM banks; each k-tile's scores go to
                        # its own bank half so one exp op covers both.
                        ps_s = attn_ps.tile([128, 2, QCH], F32, tag="ps_s",
                                            name="ps_s")
                        p_t = p_pool.tile([128, 2, QCH], MM_DT, tag="pt",
                                          name="p_t")
                        info = []
                        for idx, j in enumerate(pair):
                            # diagonal k-tiles (s >= 0): only columns
                            # [128*s, QCH) of this chunk can attend to k-tile j
                            s = j - 4 * qc
                            c0 = 128 * s if s > 0 else 0
                            info.append((j, idx, s, c0))
                            nc.tensor.matmul(
                                ps_s[:, idx, c0:QCH], kt_all[:, h, ts(j, 128)],
                                qt_all[:, h, qc * QCH + c0:(qc + 1) * QCH],
                                start=True, stop=True)
                        if len(pair) == 2 and info[0][3] == 0 and info[1][3] == 0:
                            nc.scalar.activation(p_t, ps_s,
                                                 mybir.ActivationFunctionType.Exp,
                                                 scale=float(HD) ** -0.5)
                        else:
                            for j, idx, s, c0 in info:
                                nc.scalar.activation(
                                    p_t[:, idx, c0:QCH], ps_s[:, idx, c0:QCH],
                                    mybir.ActivationFunctionType.Exp,
                                    scale=float(HD) ** -0.5)
                        for j, idx, s, c0 in info:
                            if s >= 0:
                                nc.gpsimd.tensor_mul(p_t[:, idx, c0:c0 + 128],
                                                     p_t[:, idx, c0:c0 + 128],
                                                     mask_sb)
                            nc.tensor.matmul(po[:, c0:QCH], vp_sb[:, j, h, :],
                                             p_t[:, idx, c0:QCH],
                                             start=(j == 0), stop=(j == njt - 1))
                    # normalize: 1/l, then DMA partition-broadcast of the
                    # reciprocal row (stride-0 source), then one multiply
                    recip = misc.tile([65, QCH], MM_DT, tag="recip", name="recip")
                    with nc.allow_low_precision(reason="f32r softmax denom"):
                        nc.vector.reciprocal(recip[64:65, :], po[64:65, :])
                    bcast = misc.tile([64, QCH], F32, tag="bcast", name="bcast")
                    pb = tr_ps.tile([64, QCH], F32, tag="ptr", name="pb")
                    nc.tensor.matmul(pb, ones_r[64:65, :], recip[64:65, :],
                                     start=True, stop=True)
                    nc.vector.tensor_copy(bcast, pb)
                    nc.vector.tensor_mul(yt_all[:, h, qsl], po[0:64, :], bcast)

            def cproj(qc):
                qsl = ts(qc, QCH)
                for m in range(D // 128):
                    pp = proj_ps.tile([128, QCH], F32, tag="psqk", name="pp")
                    for h in range(HPC):
                        nc.tensor.matmul(pp, wo_sb[h][:, ts(m, 128)],
                                         yt_all[:, h, qsl],
                                         start=(h == 0), stop=(h == HPC - 1))
                    ot = co_pool.tile([128, QCH], F32, tag="ot", name="ot")
                    nc.vector.tensor_copy(ot, pp)
                    nc.sync.dma_start(out=outT[ts(m, 128), qsl], in_=ot)

            def emit_body():
                for i in range(NTT):
                    if STAGES in ("all", "a"):
                        stage_a(i)
                for qc in range(NQC):
                    if STAGES in ("all", "b"):
                        attention(qc)
                        cproj(qc)

            if REPEAT > 1:
                with tc.For_i(0, REPEAT, 1):
                    emit_body()
            else:
                emit_body()

    nc.finalize()
    return nc


_NC = None


def _get_nc():
    global _NC
    if _NC is None:
        _NC = build_nc()
    return _NC


def _prep_inputs(x, wq, wk, wv, wo, alpha, cos, sin):
    x = np.asarray(x, dtype=np.float32)
    wq = np.asarray(wq, dtype=np.float32)
    wk = np.asarray(wk, dtype=np.float32)
    wv = np.asarray(wv, dtype=np.float32)
    wo = np.asarray(wo, dtype=np.float32)
    alpha = np.asarray(alpha, dtype=np.float32)
    cos = np.asarray(cos, dtype=np.float32)
    sin = np.asarray(sin, dtype=np.float32)

    # softmax over basis heads (fp32, stable)
    a = alpha - alpha.max(axis=-1, keepdims=True)
    e = np.exp(a)
    w = e / e.sum(axis=-1, keepdims=True)          # [H, KH]

    # fold the basis combination into effective per-head wk / wv
    wk_eff = np.einsum("dje,hj->dhe", wk.reshape(D, KH, HD), w).reshape(D, H * HD)
    wv_eff = np.einsum("dje,hj->dhe", wv.reshape(D, KH, HD), w).reshape(D, H * HD)

    csn = np.ascontiguousarray(np.concatenate([cos, sin], axis=1))     # [T, 64]
    scn = np.ascontiguousarray(np.concatenate([-sin, cos], axis=1))    # [T, 64]

    # single [128, 128] triangular mask (k <= q) for diagonal sub-blocks
    kk = np.arange(128)[:, None]
    qq = np.arange(128)[None, :]
    masks = np.ascontiguousarray((kk <= qq).astype(np.float32))

    in_maps = []
    for c in range(NCORES):
        b, g = c // 4, c % 4
        sl = slice(g * HPC * HD, (g + 1) * HPC * HD)
        wqkv = np.zeros((D, WQKV), dtype=np.float32)
        wqkv[:, 0:192] = wq[:, sl]
        wqkv[:, 192:384] = wk_eff[:, sl]
        wqkv[:, 384:576] = wv_eff[:, sl]
        in_maps.append({
            "xT": np.ascontiguousarray(x[b].T),
            "wqkv": wqkv,
            "wo": np.ascontiguousarray(wo[sl, :]),
            "csn": csn,
            "scn": scn,
            "masks": masks,
        })
    return in_maps


def run(trace=False, **inputs):
    nc = _get_nc()
    in_maps = _prep_inputs(**inputs)
    res = run_bass_kernel_spmd(nc, in_maps, list(range(NCORES)), trace=trace)
    out = np.zeros((B, T, D), dtype=np.float32)
    for c in range(NCORES):
        out[c // 4] += res.results[c]["outT"].T
    return out, res


def kernel(**inputs):
    out, _ = run(**inputs)
    return out

